# revision 48
# baseline (speedup 1.0000x reference)
"""AAGNN GraphConvolution kernel for 8 Trainium2 NeuronCores.

Computes relu(degree_norm * (adj @ (x @ W)) + b) for
x[16384,128], adj[16384,16384], degree_norm[16384,1], W[128,64], b[64].

Sharding: 1D row partition of the output nodes across 8 cores (2048 rows
each). Each core receives the transposed row-block of the adjacency
(adjT[16384, 2048], contiguous) so the TensorEngine can contract over the
full node axis with contiguous DMA, plus replicated xT/W/b and its
degree_norm slice. No cross-core communication is needed.

Device program per core (bf16 streams, fp32 PSUM accumulation):
  support = x @ W               (redundant on every core, [16384, 64] in SBUF)
  aggT    = support_kb-stationary matmuls over the adjT stream, two PE
            column halves computing two m-slices concurrently -> PSUM
  out     = relu(deg * aggT + b)  in a scrambled [128, 1024] layout
Host packs adjT into DMA tiles and unscrambles the outputs to [16384, 64].
"""

import sys
import types

if "/opt/trn_rl_repo" not in sys.path:
    sys.path.insert(0, "/opt/trn_rl_repo")

import numpy as np
import ml_dtypes

import concourse.bass as bass  # noqa: F401  (AP helpers)
import concourse.mybir as mybir
import concourse.tile as tile
from concourse import bacc
from concourse.bass_utils import run_bass_kernel_spmd


def _ensure_ntff_hook():
    """bass_utils imports antenv.axon_hooks when tracing is requested
    (trace=True or BASS_TRACE=1). This image's antenv lacks that module, so
    rebuild the hook from trn_agent_boot's ctypes shim — or register a None
    hook so tracing degrades gracefully instead of raising ImportError."""
    try:
        import antenv.axon_hooks  # noqa: F401

        return
    except ImportError:
        pass
    hook = None
    try:
        from trn_agent_boot.trn_boot import _ntff_profile_via_ctypes

        hook = _ntff_profile_via_ctypes("/opt/axon/libaxon_pjrt.so")
    except Exception:
        hook = None
    mod = types.ModuleType("antenv.axon_hooks")
    mod.get_axon_ntff_profile_hook = lambda: hook
    mod.set_axon_ntff_profile_hook = lambda h: None
    sys.modules["antenv.axon_hooks"] = mod


_ensure_ntff_hook()

N_NODES = 16384
F = 128  # feature size
H = 64  # hidden size
N_CORES = 8
ROWS = N_NODES // N_CORES  # 2048 output rows per core
KB = 128  # contraction block (partition dim)

# Tunables
USE_BF16 = True  # stream adjacency/x/W in bf16 (fp32 accumulation)
ADJ_BUFS = 7  # in-flight adjacency DMA tiles
KB_PER_TILE = 4  # k-blocks per adjacency DMA tile


def build_nc(
    n_nodes: int = N_NODES,
    rows: int = ROWS,
    use_bf16: bool = USE_BF16,
    adj_bufs: int = ADJ_BUFS,
    kb_per_tile: int = KB_PER_TILE,
):
    """Build the single-core Bass program (same program on every core)."""
    f32 = mybir.dt.float32
    adt = mybir.dt.bfloat16 if use_bf16 else f32
    nkb = n_nodes // KB  # number of contraction blocks
    g_size = min(8, nkb)  # k-blocks per support psum group (one bank)
    n_groups = nkb // g_size

    # Column-pairing: two concurrent matmuls on PE column halves compute two
    # different m-slices of the output. Output/deg live in a scrambled
    # [128, rows/2] layout: partition p, col i*n_slice+n  <->
    # (h = p%64, m = i*2*n_slice + (p//64)*n_slice + n); host unscrambles.
    n_slice = min(512, rows // 2)
    n_pairs = rows // (2 * n_slice)
    hcols = n_pairs * n_slice  # rows // 2

    n_tiles = nkb // kb_per_tile
    nc = bacc.Bacc("TRN2", debug=False, num_devices=N_CORES)
    # adjacency arrives host-pre-tiled: row t*128+p holds the kb_per_tile
    # k-block chunks of partition p for tile t, so each DMA tile is one
    # fully-contiguous DRAM block with 16KB-contiguous per-partition runs
    adjT = nc.declare_dram_parameter(
        "adjT", [n_tiles * KB, kb_per_tile * rows], adt, isOutput=False
    )
    xT = nc.declare_dram_parameter("xT", [F, n_nodes], adt, isOutput=False)
    Wp = nc.declare_dram_parameter("W", [F, H], adt, isOutput=False)
    bp = nc.declare_dram_parameter("b", [2 * H, 1], f32, isOutput=False)
    degp = nc.declare_dram_parameter("deg", [2 * H, hcols], f32, isOutput=False)
    outp = nc.declare_dram_parameter("out", [2 * H, hcols], f32, isOutput=True)

    with tile.TileContext(nc) as tc:
        with (
            tc.tile_pool(name="const", bufs=1) as cpool,
            tc.tile_pool(name="adj", bufs=adj_bufs) as apool,
            tc.tile_pool(name="spsum", bufs=3, space="PSUM") as spool,
            tc.tile_pool(name="accs", bufs=1, space="PSUM") as accpool,
            tc.tile_pool(name="epi", bufs=2) as epool,
        ):
            # ---- constants (ACT ring; adjacency owns the SP ring) ----
            w_sb = cpool.tile([F, H], adt, tag="w")
            nc.scalar.dma_start(out=w_sb[:], in_=Wp[:, :])
            b_sb = cpool.tile([2 * H, 1], f32, tag="b")
            nc.scalar.dma_start(out=b_sb[:], in_=bp[:, :])

            # ---- support = x @ W, stored [k partitions, h free] per k-block ----
            # xT rides the SWDGE (gpsimd) path so it starts immediately and
            # is not starved behind the adjacency stream on the HWDGE rings
            xT_sb = cpool.tile([F, n_nodes], adt, tag="xT")
            nc.gpsimd.dma_start(out=xT_sb[:], in_=xT[:, :])
            support_sb = cpool.tile([KB, nkb * H], adt, tag="support")
            for g in range(n_groups):
                # g_size k-blocks share one psum bank; one batched DVE copy
                ps = spool.tile([KB, g_size * H], f32, tag="spsum", name="ps")
                for i in range(g_size):
                    kb = g * g_size + i
                    nc.tensor.matmul(
                        out=ps[:, i * H : (i + 1) * H],
                        lhsT=xT_sb[:, kb * KB : (kb + 1) * KB],
                        rhs=w_sb[:],
                        start=(i == 0),
                        stop=(i == g_size - 1),
                        skip_group_check=True,
                    )
                nc.vector.tensor_copy(
                    out=support_sb[:, g * g_size * H : (g + 1) * g_size * H],
                    in_=ps[:],
                )

            # ---- aggregation: aggT[h, m] += support_kb.T-stationary @ adjT ----
            # Each k-block issues 2*n_pairs matmuls; within a pair the two
            # matmuls target different PE column halves (tile_position) and
            # run concurrently, computing two different m-slices.
            accs = [
                accpool.tile([2 * H, n_slice], f32, tag=f"acc{i}", name=f"acc{i}")
                for i in range(n_pairs)
            ]
            half = kb_per_tile * rows // 2
            for t in range(n_tiles):
                a = apool.tile([KB, kb_per_tile * rows], adt, tag="adj", name="a")
                # adjacency owns the SP HWDGE ring (no ACT_TABLE_LOAD ahead
                # of it) and is one contiguous DRAM block per tile. Only the
                # final tile is split in two so its first matmuls overlap the
                # second half's transfer, trimming the kernel tail.
                if t == n_tiles - 1 and kb_per_tile > 1:
                    nc.sync.dma_start(
                        out=a[:, :half], in_=adjT[t * KB : (t + 1) * KB, :half]
                    )
                    nc.sync.dma_start(
                        out=a[:, half:], in_=adjT[t * KB : (t + 1) * KB, half:]
                    )
                else:
                    nc.sync.dma_start(
                        out=a[:], in_=adjT[t * KB : (t + 1) * KB, :]
                    )
                for j in range(kb_per_tile):
                    kb = t * kb_per_tile + j
                    sup = support_sb[:, kb * H : (kb + 1) * H]
                    for i in range(n_pairs):
                        for u in (0, 1):
                            m0 = (2 * i + u) * n_slice
                            nc.tensor.matmul(
                                out=accs[i][u * H : (u + 1) * H, :],
                                lhsT=sup,
                                rhs=a[:, j * rows + m0 : j * rows + m0 + n_slice],
                                start=(kb == 0),
                                stop=(kb == nkb - 1),
                                tile_position=(0, u * H),
                                # the two column halves are disjoint partition
                                # groups in the same bank; the coarse zero-region
                                # group check can't express that
                                skip_group_check=True,
                            )

            # ---- epilogue: relu(deg * aggT + b), in the scrambled layout ----
            deg_sb = cpool.tile([2 * H, hcols], f32, tag="deg")
            nc.gpsimd.dma_start(out=deg_sb[:], in_=degp[:, :])
            o_sb = epool.tile([2 * H, hcols], f32, tag="o", name="o")
            for i in range(n_pairs):
                tmp = epool.tile([2 * H, n_slice], f32, tag="tmp", name="tmp")
                nc.vector.tensor_tensor(
                    out=tmp[:],
                    in0=accs[i][:],
                    in1=deg_sb[:, i * n_slice : (i + 1) * n_slice],
                    op=mybir.AluOpType.mult,
                )
                nc.scalar.activation(
                    out=o_sb[:, i * n_slice : (i + 1) * n_slice],
                    in_=tmp[:],
                    func=mybir.ActivationFunctionType.Relu,
                    bias=b_sb[:],
                )
                # per-pair output DMA overlaps the other pair's epilogue
                nc.scalar.dma_start(
                    out=outp[:, i * n_slice : (i + 1) * n_slice],
                    in_=o_sb[:, i * n_slice : (i + 1) * n_slice],
                )

    nc.compile()
    return nc


def pack_adjT(adjT_c, rows, kb_per_tile=KB_PER_TILE):
    """[n_nodes, rows] transposed adjacency shard -> DMA-tiled layout
    [n_tiles*128, kb_per_tile*rows]: row t*128+p concatenates the
    kb_per_tile k-block rows (4t+j)*128+p, giving contiguous per-partition
    runs inside each 2 MiB tile."""
    n_nodes = adjT_c.shape[0]
    n_tiles = n_nodes // (KB * kb_per_tile)
    return np.ascontiguousarray(
        adjT_c.reshape(n_tiles, kb_per_tile, KB, rows)
        .transpose(0, 2, 1, 3)
        .reshape(n_tiles * KB, kb_per_tile * rows)
    )


def scramble_cols(v, rows):
    """[rows] vector -> [128, rows//2] scrambled layout matching the kernel:
    out[p, i*ns+n] = v[i*2*ns + (p//64)*ns + n]."""
    ns = min(512, rows // 2)
    npair = rows // (2 * ns)
    m = v.reshape(npair, 2, ns)  # [i, u, n]
    out = np.empty((2 * H, npair * ns), dtype=v.dtype)
    for u in (0, 1):
        half = m[:, u, :].reshape(1, npair * ns)
        out[u * H : (u + 1) * H, :] = np.broadcast_to(half, (H, npair * ns))
    return out


def unscramble_out(o, rows):
    """[128, rows//2] kernel output -> [rows, H] natural layout."""
    ns = min(512, rows // 2)
    npair = rows // (2 * ns)
    outT = np.empty((H, rows), dtype=o.dtype)
    for i in range(npair):
        for u in (0, 1):
            outT[:, (2 * i + u) * ns : (2 * i + u + 1) * ns] = o[
                u * H : (u + 1) * H, i * ns : (i + 1) * ns
            ]
    return outT.T


def make_in_maps(x, adj_matrix, degree_norm, W, b, use_bf16=USE_BF16):
    """Shard the full inputs into per-core input maps (host-side, numpy)."""
    adt = ml_dtypes.bfloat16 if use_bf16 else np.float32
    xT = np.ascontiguousarray(x.T, dtype=adt)
    Wf = np.ascontiguousarray(W, dtype=adt)
    bf = np.ascontiguousarray(
        np.concatenate([np.asarray(b, np.float32)] * 2), dtype=np.float32
    ).reshape(2 * H, 1)
    in_maps = []
    for c in range(N_CORES):
        r0, r1 = c * ROWS, (c + 1) * ROWS
        adjT_c = pack_adjT(
            np.ascontiguousarray(adj_matrix[r0:r1, :].T, dtype=adt), ROWS
        )
        deg_c = scramble_cols(
            np.ascontiguousarray(degree_norm[r0:r1].reshape(-1), np.float32), ROWS
        )
        in_maps.append({"adjT": adjT_c, "xT": xT, "W": Wf, "b": bf, "deg": deg_c})
    return in_maps


_nc_cache = {}


def _get_nc():
    key = (USE_BF16, ADJ_BUFS, KB_PER_TILE)
    if key not in _nc_cache:
        _nc_cache[key] = build_nc()
    return _nc_cache[key]


def kernel(x, adj_matrix, degree_norm, W, b):
    x = np.asarray(x)
    adj_matrix = np.asarray(adj_matrix)
    degree_norm = np.asarray(degree_norm)
    W = np.asarray(W)
    b = np.asarray(b)

    nc = _get_nc()
    in_maps = make_in_maps(x, adj_matrix, degree_norm, W, b)
    try:
        res = run_bass_kernel_spmd(nc, in_maps, core_ids=list(range(N_CORES)))
    except Exception:
        # transient NRT_EXEC_UNIT_UNRECOVERABLE after an aborted prior run
        # heals after touching the devices once; retry a single time
        try:
            import jax, jax.numpy as jnp  # noqa: E401

            for d in jax.devices():
                jnp.add(jax.device_put(jnp.ones((2, 2)), d), 1.0).block_until_ready()
        except Exception:
            pass
        res = run_bass_kernel_spmd(nc, in_maps, core_ids=list(range(N_CORES)))
    out = np.empty((N_NODES, H), dtype=np.float32)
    for c in range(N_CORES):
        out[c * ROWS : (c + 1) * ROWS, :] = unscramble_out(res.results[c]["out"], ROWS)
    return out
